# revision 1
# baseline (speedup 1.0000x reference)
"""AttnBlock (B=2, C=512, H=W=64) on 8 TRN2 NeuronCores.

Sharding: core c handles batch b=c//4 and query-quarter q=c%4 (1024 of 4096
query positions). Keys/values are computed redundantly per core from the
full batch image (group-norm needs all of it anyway). The key axis is
host-permuted per core so the core's query quarter occupies columns 0:1024
of its buffer — softmax/attention are permutation-invariant over keys, so
the same SPMD program works on every core with no dynamic indexing.

Attention is computed via S^T = k^T q (keys stationary): softmax runs
without max-subtraction (logits are ~N(0,1), exp is safe in fp32), the
exp(S^T) tiles feed the PV matmul directly as stationary operand, row sums
come from a ones-vector matmul, and 1/Z is folded into a final
per-partition scale.

Matmuls run in float32r (fp32 rounded to 11 mantissa bits, full PE rate).
Weights are pre-rounded on the host; on-device producers write f32r so the
PE consumes rounded values. The residual path stays exact fp32.
"""

import numpy as np

import concourse.bass as bass
import concourse.tile as tile
from concourse import bacc, mybir
from concourse.bass_utils import run_bass_kernel_spmd

F32 = mybir.dt.float32
F32R = mybir.dt.float32r

P = 128          # partitions
CT = 4           # channel tiles (C = 512 = 4*128)
C = 512
N = 4096         # H*W
NS = 8           # 512-wide column slices of N
NJT = 32         # 128-wide key tiles
NQ = 1024        # query columns per core
B = 2
HW = 64
NGROUPS = 32
GSIZE = C // NGROUPS  # 16 channels per group
EPS = 1e-5
SCL = float(C) ** -0.5
NCORES = 8

_cached = {}


def _round_f32r(a):
    """Round fp32 to 11 mantissa bits (RNE), keep fp32 container."""
    u = np.ascontiguousarray(a, dtype=np.float32).view(np.uint32)
    keep = np.uint32(0xFFFFF000)
    bias = np.uint32(0x800) - ((u >> np.uint32(12)) & np.uint32(1))
    return ((u + bias) & keep).view(np.float32)


def _ct_layout(v):
    """[C] -> [P, CT] with channel c at [c % 128, c // 128]."""
    return np.ascontiguousarray(v.reshape(CT, P).T, dtype=np.float32)


def _cmaj(a2d, ncols):
    """[C, ncols] -> [P, CT, ncols]."""
    return np.ascontiguousarray(
        a2d.reshape(CT, P, ncols).transpose(1, 0, 2), dtype=np.float32
    )


def _build_program():
    nc = bacc.Bacc("TRN2", target_bir_lowering=False, debug=False)

    X_d = nc.declare_dram_parameter("xin", [P, CT, N], F32R, isOutput=False)
    XQ_d = nc.declare_dram_parameter("xq", [P, CT, NQ], F32, isOutput=False)
    WQ_d = nc.declare_dram_parameter("wqt", [P, CT, C], F32R, isOutput=False)
    WK_d = nc.declare_dram_parameter("wkt", [P, CT, C], F32R, isOutput=False)
    WV_d = nc.declare_dram_parameter("wvt", [P, CT, C], F32R, isOutput=False)
    WP_d = nc.declare_dram_parameter("wpt", [P, CT, C], F32R, isOutput=False)
    BQ_d = nc.declare_dram_parameter("bq2", [P, CT], F32, isOutput=False)
    BK_d = nc.declare_dram_parameter("bk2", [P, CT], F32, isOutput=False)
    BPE_d = nc.declare_dram_parameter("bpe", [P, CT], F32, isOutput=False)
    GAM_d = nc.declare_dram_parameter("gam", [P, CT], F32, isOutput=False)
    BET_d = nc.declare_dram_parameter("bet", [P, CT], F32, isOutput=False)
    G_d = nc.declare_dram_parameter("gmat", [P, CT, NGROUPS], F32, isOutput=False)
    E_d = nc.declare_dram_parameter("emat", [NGROUPS, CT, P], F32, isOutput=False)
    ID_d = nc.declare_dram_parameter("ident", [P, P], F32, isOutput=False)
    ONE_d = nc.declare_dram_parameter("ones1", [P, 1], F32, isOutput=False)
    OF_d = nc.declare_dram_parameter("onef", [1, 1], F32, isOutput=False)
    OUT_d = nc.declare_dram_parameter("out", [P, CT, NQ], F32, isOutput=True)

    with tile.TileContext(nc) as tc:
        with (
            tc.tile_pool(name="big", bufs=1) as big,
            tc.tile_pool(name="consts", bufs=1) as consts,
            tc.tile_pool(name="stat", bufs=1) as stat,
        ):
            X = big.tile([P, CT, N], F32R)
            VT = big.tile([P, NJT, C], F32R)
            QO = big.tile([P, CT, NQ], F32R)
            SPARE = big.tile([P, CT, 512], F32R)

            wp = consts.tile([P, CT, C], F32R)
            bpe_sb = consts.tile([P, CT], F32)
            bq_sb = consts.tile([P, CT], F32)
            bk_sb = consts.tile([P, CT], F32)
            gam_sb = consts.tile([P, CT], F32)
            bet_sb = consts.tile([P, CT], F32)
            gmat = consts.tile([P, CT, NGROUPS], F32)
            emat = consts.tile([NGROUPS, CT, P], F32)
            ident = consts.tile([P, P], F32)
            ones1 = consts.tile([P, 1], F32)
            onef = consts.tile([1, 1], F32)

            nc.sync.dma_start(out=ident, in_=ID_d[:])
            for s in range(NS):
                sl = slice(s * 512, (s + 1) * 512)
                nc.sync.dma_start(out=X[:, :, sl], in_=X_d[:, :, sl])
            nc.sync.dma_start(out=gmat, in_=G_d[:])
            nc.sync.dma_start(out=emat, in_=E_d[:])
            nc.sync.dma_start(out=gam_sb, in_=GAM_d[:])
            nc.sync.dma_start(out=bet_sb, in_=BET_d[:])
            nc.sync.dma_start(out=bq_sb, in_=BQ_d[:])
            nc.sync.dma_start(out=bk_sb, in_=BK_d[:])
            nc.sync.dma_start(out=ones1, in_=ONE_d[:])
            nc.sync.dma_start(out=onef, in_=OF_d[:])

            # ---------------- Phase 1: group-norm statistics ----------------
            bnst = stat.tile([P, CT, NS, 6], F32)
            for s in range(NS):
                for t in range(CT):
                    nc.vector.bn_stats(
                        out=bnst[:, t, s, :],
                        in_=X[:, t, s * 512 : (s + 1) * 512].bitcast(F32),
                    )
            mex = stat.tile([P, CT, 2], F32)
            for t in range(CT):
                nc.vector.bn_aggr(out=mex[:, t, :], in_=bnst[:, t, :, :])
            # mexp[...,0] = mean, mexp[...,1] = E[x^2] = var + mean^2
            mexp = stat.tile([P, CT, 2], F32)
            nc.vector.tensor_copy(out=mexp[:, :, 0], in_=mex[:, :, 0])
            nc.vector.tensor_tensor(
                out=mexp[:, :, 1], in0=mex[:, :, 0], in1=mex[:, :, 0],
                op=mybir.AluOpType.mult,
            )
            nc.vector.tensor_add(
                out=mexp[:, :, 1], in0=mexp[:, :, 1], in1=mex[:, :, 1]
            )

            scale_c = stat.tile([P, CT], F32)
            shift_c = stat.tile([P, CT], F32)
            with tc.tile_pool(name="psum_p1", bufs=1, space="PSUM") as p1:
                gs_ps = p1.tile([NGROUPS, 2], F32, tag="gs")
                for t in range(CT):
                    nc.tensor.matmul(
                        gs_ps, gmat[:, t, :], mexp[:, t, :],
                        start=(t == 0), stop=(t == CT - 1),
                    )
                gsb = stat.tile([NGROUPS, 2], F32)
                nc.vector.tensor_copy(out=gsb, in_=gs_ps)
                gmr = stat.tile([NGROUPS, 2], F32)
                gtmp = stat.tile([NGROUPS, 2], F32)
                nc.scalar.mul(out=gmr[:, 0:1], in_=gsb[:, 0:1], mul=1.0 / GSIZE)
                nc.scalar.mul(out=gtmp[:, 0:1], in_=gsb[:, 1:2], mul=1.0 / GSIZE)
                nc.vector.tensor_tensor(
                    out=gtmp[:, 1:2], in0=gmr[:, 0:1], in1=gmr[:, 0:1],
                    op=mybir.AluOpType.mult,
                )
                nc.vector.tensor_sub(
                    out=gtmp[:, 0:1], in0=gtmp[:, 0:1], in1=gtmp[:, 1:2]
                )
                eps_sb = stat.tile([NGROUPS, 1], F32)
                nc.vector.memset(eps_sb, EPS)
                nc.scalar.activation(
                    out=gtmp[:, 0:1], in_=gtmp[:, 0:1],
                    func=mybir.ActivationFunctionType.Sqrt, bias=eps_sb,
                )
                nc.vector.reciprocal(out=gmr[:, 1:2], in_=gtmp[:, 0:1])
                mc = stat.tile([P, CT, 2], F32)
                for t in range(CT):
                    ms_ps = p1.tile([P, 2], F32, tag="ms")
                    nc.tensor.matmul(ms_ps, emat[:, t, :], gmr, start=True, stop=True)
                    nc.vector.tensor_copy(out=mc[:, t, :], in_=ms_ps)
                nc.vector.tensor_tensor(
                    out=scale_c, in0=mc[:, :, 1], in1=gam_sb, op=mybir.AluOpType.mult
                )
                nc.vector.tensor_tensor(
                    out=shift_c, in0=mc[:, :, 0], in1=scale_c, op=mybir.AluOpType.mult
                )
                nc.vector.tensor_sub(out=shift_c, in0=bet_sb, in1=shift_c)

            # ---------------- Phase 2: normalize + q/k/vT projections -------
            def norm_slice(s):
                sl = slice(s * 512, (s + 1) * 512)
                for t in range(CT):
                    nc.vector.tensor_scalar(
                        out=X[:, t, sl],
                        in0=X[:, t, sl].bitcast(F32),
                        scalar1=scale_c[:, t : t + 1],
                        scalar2=shift_c[:, t : t + 1],
                        op0=mybir.AluOpType.mult,
                        op1=mybir.AluOpType.add,
                    )

            with (
                tc.tile_pool(name="wqkv", bufs=1) as wpool,
                tc.tile_pool(name="psum2", bufs=1, space="PSUM") as psum2,
            ):
                wq = wpool.tile([P, CT, C], F32R)
                wk = wpool.tile([P, CT, C], F32R)
                wv = wpool.tile([P, CT, C], F32R)
                nc.sync.dma_start(out=wq, in_=WQ_d[:])
                nc.sync.dma_start(out=wk, in_=WK_d[:])
                nc.sync.dma_start(out=wv, in_=WV_d[:])
                nc.sync.dma_start(out=wp, in_=WP_d[:])
                nc.sync.dma_start(out=bpe_sb, in_=BPE_d[:])

                norm_slice(0)
                for s in range(NS):
                    if s + 1 < NS:
                        norm_slice(s + 1)
                    sl = slice(s * 512, (s + 1) * 512)
                    if s < 2:
                        for ct in range(CT):
                            qp = psum2.tile([P, 512], F32, tag="acc", bufs=3)
                            for kt in range(CT):
                                nc.tensor.matmul(
                                    qp,
                                    wq[:, kt, ct * P : (ct + 1) * P],
                                    X[:, kt, sl],
                                    start=(kt == 0), stop=(kt == CT - 1),
                                )
                            nc.scalar.activation(
                                out=QO[:, ct, s * 512 : (s + 1) * 512], in_=qp,
                                func=mybir.ActivationFunctionType.Identity,
                                bias=bq_sb[:, ct : ct + 1],
                            )
                    for jt in range(CT):
                        vp = psum2.tile([P, 512], F32, tag="acc", bufs=3)
                        jcol = slice(s * 512 + jt * P, s * 512 + (jt + 1) * P)
                        for kt in range(CT):
                            nc.tensor.matmul(
                                vp, X[:, kt, jcol], wv[:, kt, :],
                                start=(kt == 0), stop=(kt == CT - 1),
                            )
                        nc.vector.tensor_copy(out=VT[:, s * 4 + jt, :], in_=vp)
                    # k overwrites the previous (dead) slice region; k(0)->SPARE
                    for ct in range(CT):
                        kp = psum2.tile([P, 512], F32, tag="acc", bufs=3)
                        for kt in range(CT):
                            nc.tensor.matmul(
                                kp,
                                wk[:, kt, ct * P : (ct + 1) * P],
                                X[:, kt, sl],
                                start=(kt == 0), stop=(kt == CT - 1),
                            )
                        if s == 0:
                            kdst = SPARE[:, ct, :]
                        else:
                            kdst = X[:, ct, (s - 1) * 512 : s * 512]
                        nc.scalar.activation(
                            out=kdst, in_=kp,
                            func=mybir.ActivationFunctionType.Identity,
                            bias=bk_sb[:, ct : ct + 1],
                        )

            # ---------------- Phase 3: attention (S^T route) -----------------
            def key_block(jt, kt):
                """[128 c, 128 j] block of keys for global key tile jt."""
                js, sub = jt // 4, jt % 4
                if js == 0:
                    return SPARE[:, kt, sub * P : (sub + 1) * P]
                base = (js - 1) * 512 + sub * P
                return X[:, kt, base : base + P]

            with (
                tc.tile_pool(name="psum3", bufs=1, space="PSUM") as psum3,
                tc.tile_pool(name="pwork", bufs=1) as pwork,
            ):
                deferred = []

                def pop_deferred():
                    if deferred:
                        deferred.pop(0)()

                def st_group(isl, jt):
                    """S^T matmuls + exp for key tile jt against i-slice isl."""
                    s_ps = psum3.tile([P, 512], F32, tag="s", bufs=2)
                    isl_sl = slice(isl * 512, (isl + 1) * 512)
                    for kt in range(CT):
                        nc.tensor.matmul(
                            s_ps,
                            key_block(jt, kt),
                            QO[:, kt, isl_sl],
                            start=(kt == 0), stop=(kt == CT - 1),
                        )
                    pt = pwork.tile([P, 512], F32R, tag="p", bufs=4)
                    nc.scalar.activation(
                        out=pt, in_=s_ps,
                        func=mybir.ActivationFunctionType.Exp, scale=SCL,
                    )
                    return pt

                def emit_znorm(isl, zsum, u_list):
                    """Normalize u blocks by 1/Z immediately (frees u banks)."""
                    z_ps = psum3.tile([1, 512], F32, tag="t", bufs=2)
                    nc.tensor.matmul(z_ps, ones1, zsum, start=True, stop=True)
                    zrow = pwork.tile([1, 512], F32, tag="zrow", bufs=2)
                    nc.vector.tensor_copy(out=zrow, in_=z_ps)
                    nc.vector.reciprocal(out=zrow, in_=zrow)
                    osbs = []
                    for ib in range(4):
                        zx_ps = psum3.tile([P, 1], F32, tag="t", bufs=2)
                        nc.tensor.matmul(
                            zx_ps, zrow[:, ib * P : (ib + 1) * P], onef,
                            start=True, stop=True,
                        )
                        zinv = pwork.tile([P, 1], F32, tag="zinv", bufs=2)
                        nc.vector.tensor_copy(out=zinv, in_=zx_ps)
                        osb = pwork.tile([P, C], F32R, tag="osb", bufs=4)
                        nc.vector.tensor_scalar_mul(
                            out=osb, in0=u_list[ib], scalar1=zinv
                        )
                        osbs.append(osb)
                    return osbs

                def otr_closures(isl, osbs):
                    """Deferred: transpose normalized O^T blocks into QO."""
                    ops = []
                    for ib in range(4):
                        for ct in range(CT):
                            def otr(ib=ib, ct=ct):
                                t_ps = psum3.tile([P, P], F32, tag="t", bufs=2)
                                nc.tensor.transpose(
                                    t_ps,
                                    osbs[ib][:, ct * P : (ct + 1) * P].bitcast(F32),
                                    ident,
                                )
                                nc.vector.tensor_copy(
                                    out=QO[:, ct, isl * 512 + ib * P : isl * 512 + (ib + 1) * P],
                                    in_=t_ps,
                                )

                            ops.append(otr)
                    return ops

                def proj_group(h, ct):
                    """Projection + bias + residual + store for one 128x512
                    output block. Requires O (QO cols of i-slice h) final."""
                    sl = slice(h * 512, (h + 1) * 512)
                    pr = psum3.tile([P, 512], F32, tag="s", bufs=2)
                    for kt in range(CT):
                        nc.tensor.matmul(
                            pr,
                            wp[:, kt, ct * P : (ct + 1) * P],
                            QO[:, kt, sl],
                            start=(kt == 0), stop=(kt == CT - 1),
                        )
                    xqt = pwork.tile([P, 512], F32, tag="xqt", bufs=3)
                    nc.sync.dma_start(out=xqt, in_=XQ_d[:, ct, sl])
                    ost = pwork.tile([P, 512], F32, tag="ost", bufs=3)
                    nc.vector.scalar_tensor_tensor(
                        out=ost, in0=pr, scalar=bpe_sb[:, ct : ct + 1],
                        in1=xqt, op0=mybir.AluOpType.add,
                        op1=mybir.AluOpType.add,
                    )
                    nc.sync.dma_start(out=OUT_d[:, ct, sl], in_=ost)

                for isl in range(2):
                    zsum = pwork.tile([P, 512], F32, tag="zsum", bufs=2)
                    u_list = [
                        psum3.tile([P, C], F32, tag=f"u{ib}", bufs=1, name=f"u{ib}")
                        for ib in range(4)
                    ]
                    cur_pt = st_group(isl, 0)
                    for jt in range(NJT):
                        if jt + 1 < NJT:
                            nxt_pt = st_group(isl, jt + 1)
                        if jt == 0:
                            nc.vector.tensor_copy(out=zsum, in_=cur_pt.bitcast(F32))
                        else:
                            nc.vector.tensor_add(
                                out=zsum, in0=zsum, in1=cur_pt.bitcast(F32)
                            )
                        for ib in range(4):
                            nc.tensor.matmul(
                                u_list[ib],
                                cur_pt[:, ib * P : (ib + 1) * P],
                                VT[:, jt, :],
                                start=(jt == 0), stop=(jt == NJT - 1),
                            )
                        pop_deferred()
                        # i-slice 0's O is final once its 16 transposes popped
                        # (by jt=15 of isl 1) — run the h=0 projection here.
                        if isl == 1 and jt >= 17 and (jt - 17) % 4 == 0:
                            proj_group(0, (jt - 17) // 4)
                        if jt + 1 < NJT:
                            cur_pt = nxt_pt
                    osbs = emit_znorm(isl, zsum, u_list)
                    deferred.extend(otr_closures(isl, osbs))

                # ---------------- Phase 4: remaining projection (h=1) --------
                # i-slice 1's O-transposes must fully drain before h=1 emits
                # (emission order defines the dependency graph).
                while deferred:
                    pop_deferred()
                for ct in range(CT):
                    proj_group(1, ct)

    nc.compile()
    return nc


def _get_nc():
    if "nc" not in _cached:
        _cached["nc"] = _build_program()
    return _cached["nc"]


def _make_in_maps(x, norm_gamma, norm_beta, wq, bq, wk, bk, wv, bv, wp, bp):
    gm = np.zeros((P, CT, NGROUPS), np.float32)
    em = np.zeros((NGROUPS, CT, P), np.float32)
    for t in range(CT):
        for p in range(P):
            g = (t * P + p) // GSIZE
            gm[p, t, g] = 1.0
            em[g, t, p] = 1.0

    common = {
        "wqt": _round_f32r(_cmaj(np.asarray(wq).T, C)),
        "wkt": _round_f32r(_cmaj(np.asarray(wk).T, C)),
        "wvt": _round_f32r(_cmaj(np.asarray(wv).T, C)),
        "wpt": _round_f32r(_cmaj(np.asarray(wp).T, C)),
        "bq2": _ct_layout(np.asarray(bq)),
        "bk2": _ct_layout(np.asarray(bk)),
        "bpe": _ct_layout(np.asarray(bp) + np.asarray(wp) @ np.asarray(bv)),
        "gam": _ct_layout(np.asarray(norm_gamma)),
        "bet": _ct_layout(np.asarray(norm_beta)),
        "gmat": gm,
        "emat": em,
        "ident": np.eye(P, dtype=np.float32),
        "ones1": np.ones((P, 1), np.float32),  # fp32 (exact) reducer vector
        "onef": np.ones((1, 1), np.float32),
    }

    in_maps = []
    for c in range(NCORES):
        b, qi = c // 4, c % 4
        xb = np.asarray(x[b], dtype=np.float32).reshape(C, N)
        xp = np.concatenate([xb[:, qi * NQ :], xb[:, : qi * NQ]], axis=1)
        m = dict(common)
        m["xin"] = _round_f32r(_cmaj(xp, N))
        m["xq"] = _cmaj(xb[:, qi * NQ : (qi + 1) * NQ], NQ)
        in_maps.append(m)
    return in_maps


def _assemble(results):
    out = np.empty((B, C, N), np.float32)
    for c in range(NCORES):
        b, qi = c // 4, c % 4
        r = results[c]["out"]  # [P, CT, NQ]
        out[b, :, qi * NQ : (qi + 1) * NQ] = (
            r.transpose(1, 0, 2).reshape(C, NQ)
        )
    return out.reshape(B, C, HW, HW)


def _run(inputs, trace=False, trace_kwargs=None):
    nc = _get_nc()
    in_maps = _make_in_maps(**inputs)
    res = run_bass_kernel_spmd(
        nc, in_maps, list(range(NCORES)), trace=trace,
        **(trace_kwargs or {}),
    )
    return res


def kernel(**inputs):
    res = _run(inputs)
    return _assemble(res.results)



# revision 2
# speedup vs baseline: 1.0338x; 1.0338x over previous
"""AttnBlock (B=2, C=512, H=W=64) on 8 TRN2 NeuronCores — fp8 DoubleRow.

Sharding: core c handles batch b=c//4 and query-quarter q=c%4 (1024 of 4096
query positions). Keys/values are computed redundantly per core from the
full batch image; the key axis is host-permuted per core so the core's
query quarter occupies columns 0:1024 (softmax is permutation-invariant
over keys).

All matmuls run in fp8e4 (TRN FP8_EXP4, max 240) with DoubleRow perf mode:
each instruction contracts K=256 (two 128-row subtiles interleaved in the
free dim) in the same 512 cycles a bf16/f32r K=128 matmul takes — 2x.
Channel dim C=512 is stored pair-split as [128 p, 2 g, 2 t] with
c = p + 128*t + 256*g; a DR matmul contracts (p, t) for fixed g, and the
two g-groups accumulate in PSUM.

GroupNorm: bn_stats on the fp8 x (24 slices on DVE, 8 slices via ACT
Identity/Square accum_out), group reduce via one-hot matmuls; scale/shift
fold into the q/k/v weights on device (wq' = wq*diag(s), bias' = wq@t + b
via tiny DR matvecs), so no separate normalize pass exists. Softmax runs
unshifted with exp(s*C^-.5 - 2) to keep fp8 in range; Z comes from a
ones-stationary DR matmul accumulated alongside PV; O is normalized by
1/Z (broadcast via a K=1 outer-product matmul + full-width reciprocal)
during PSUM evacuation. The residual path stays exact fp32.
"""

import numpy as np
import ml_dtypes

import concourse.bass as bass
import concourse.tile as tile
from concourse import bacc, mybir
from concourse.bass_utils import run_bass_kernel_spmd

F32 = mybir.dt.float32
F32R = mybir.dt.float32r
F8 = mybir.dt.float8e4
E4 = ml_dtypes.float8_e4m3
DR = mybir.MatmulPerfMode.DoubleRow
AF = mybir.ActivationFunctionType

P = 128
C = 512
N = 4096          # H*W keys
NQ = 1024         # query columns per core
NS = 8            # 512-wide column slices of N
SPL = 6           # slices per (g,t) whose stats run on DVE (rest on ACT)
NJP = 16          # 256-wide key pair-tiles
B = 2
HW = 64
NGROUPS = 32
GSIZE = C // NGROUPS
EPS = 1e-5
SCL = float(C) ** -0.5
EBIAS = -2.0      # exp(s*SCL - 2): max logit ~5.5 -> exp(3.5)=33 << 240
TS = 64.0         # shift vector pre-scale for fp8 matvec
NCORES = 8

_cached = {}


def _build_program():
    nc = bacc.Bacc("TRN2", target_bir_lowering=False, debug=False)

    X8_d = nc.declare_dram_parameter("xin8", [P, 2, 2, N], F8, isOutput=False)
    W3_d = nc.declare_dram_parameter("w38", [P, 2, 2, 3, C], F8, isOutput=False)
    WP_d = nc.declare_dram_parameter("wp8", [P, 2, 2, C], F8, isOutput=False)
    # packed per-channel f32 consts: bq, bk, bv, bp, gamma, beta
    CP_d = nc.declare_dram_parameter("cpack", [P, 2, 2, 6], F32, isOutput=False)
    G_d = nc.declare_dram_parameter("gmat", [P, 2, 2, NGROUPS], F32, isOutput=False)
    E_d = nc.declare_dram_parameter("emat", [NGROUPS, 2, 2, P], F32, isOutput=False)
    ON8_d = nc.declare_dram_parameter("ones8", [P, 2, P], F8, isOutput=False)
    ONQ_d = nc.declare_dram_parameter("onesq", [1, P], F32R, isOutput=False)
    XQ_d = nc.declare_dram_parameter("xq", [P, 2, 2, NQ], F32, isOutput=False)
    OUT_d = nc.declare_dram_parameter("out", [P, 2, 2, NQ], F32, isOutput=True)

    with tile.TileContext(nc) as tc:
        with (
            tc.tile_pool(name="big", bufs=1) as big,
            tc.tile_pool(name="consts", bufs=1) as consts,
            tc.tile_pool(name="stat", bufs=1) as stat,
            tc.tile_pool(name="work", bufs=1) as work,
        ):
            X8 = big.tile([P, 2, 2, N], F8)
            K8 = big.tile([P, 2, 2, N], F8)
            VT8 = big.tile([P, NJP, 2, C], F8)
            QO8 = big.tile([P, 2, 2, NQ], F8)
            xq_sb = big.tile([P, 2, 2, NQ], F32)
            ost_sb = big.tile([P, 2, 2, NQ], F32)

            w38 = consts.tile([P, 2, 2, 3, C], F8)
            w3f = consts.tile([P, 2, 2, 3, C], F8)
            wp8 = consts.tile([P, 2, 2, C], F8)
            cpk = consts.tile([P, 2, 2, 6], F32)
            gmat = consts.tile([P, 2, 2, NGROUPS], F32)
            emat = consts.tile([NGROUPS, 2, 2, P], F32)
            on8 = consts.tile([P, 2, P], F8)
            onq = consts.tile([1, P], F32R)

            # preload ACT tables (Identity/Square/Sqrt/Exp) while DMA runs
            dummy = stat.tile([1, 2], F32)
            nc.vector.memset(dummy, 1.0)
            dscr = stat.tile([1, 2], F32)
            for fn in (AF.Identity, AF.Square, AF.Sqrt, AF.Exp):
                nc.scalar.activation(out=dscr, in_=dummy, func=fn)

            # x first: groupnorm stats are the serial head of the kernel
            for g in range(2):
                for t2 in range(2):
                    nc.sync.dma_start(out=X8[:, g, t2, :], in_=X8_d[:, g, t2, :])
            for t_ in (
                (w38, W3_d), (cpk, CP_d),
                (gmat, G_d), (emat, E_d), (on8, ON8_d), (onq, ONQ_d),
                (wp8, WP_d),
            ):
                nc.sync.dma_start(out=t_[0], in_=t_[1][:])
            nc.sync.dma_start(out=xq_sb, in_=XQ_d[:])

            bqh = cpk[:, :, :, 0]
            bkh = cpk[:, :, :, 1]
            bvh = cpk[:, :, :, 2]
            bph = cpk[:, :, :, 3]
            gam_sb = cpk[:, :, :, 4]
            bet_sb = cpk[:, :, :, 5]

            # ---------------- Phase 1: group-norm statistics ----------------
            # 24 slices via DVE bn_stats, 8 slices via ACT accum (sum, sumsq)
            bnst = stat.tile([P, 2, 2, SPL, 6], F32)
            asum = stat.tile([P, 2, 2, 2, 2], F32)
            ascr = stat.tile([P, 2, 512], F8)
            mex = stat.tile([P, 2, 2, 2], F32)
            for g in range(2):
                for t2 in range(2):
                    for s in range(SPL):
                        nc.vector.bn_stats(
                            out=bnst[:, g, t2, s, :],
                            in_=X8[:, g, t2, s * 512 : (s + 1) * 512],
                        )
                    nc.vector.bn_aggr(
                        out=mex[:, g, t2, :], in_=bnst[:, g, t2, :, :]
                    )
                    for si in range(2):
                        sl = slice((SPL + si) * 512, (SPL + si + 1) * 512)
                        nc.scalar.activation(
                            out=ascr[:, 0, :], in_=X8[:, g, t2, sl],
                            func=AF.Identity,
                            accum_out=asum[:, g, t2, si, 0:1],
                        )
                        nc.scalar.activation(
                            out=ascr[:, 1, :], in_=X8[:, g, t2, sl],
                            func=AF.Square,
                            accum_out=asum[:, g, t2, si, 1:2],
                        )
            # mexp[...,0] = mean over 4096, mexp[...,1] = E[x^2] over 4096
            W_DVE = SPL / float(NS)
            astot = stat.tile([P, 2, 2, 2], F32)
            nc.vector.tensor_add(
                out=astot, in0=asum[:, :, :, 0, :], in1=asum[:, :, :, 1, :]
            )
            mexp = stat.tile([P, 2, 2, 2], F32)
            t1s = stat.tile([P, 2, 2], F32)
            nc.vector.tensor_scalar(
                out=t1s, in0=mex[:, :, :, 0], scalar1=W_DVE, scalar2=None,
                op0=mybir.AluOpType.mult,
            )
            nc.vector.scalar_tensor_tensor(
                out=mexp[:, :, :, 0], in0=astot[:, :, :, 0],
                scalar=1.0 / float(N), in1=t1s,
                op0=mybir.AluOpType.mult, op1=mybir.AluOpType.add,
            )
            nc.vector.tensor_tensor(
                out=t1s, in0=mex[:, :, :, 0], in1=mex[:, :, :, 0],
                op=mybir.AluOpType.mult,
            )
            nc.vector.tensor_add(out=t1s, in0=t1s, in1=mex[:, :, :, 1])
            nc.vector.tensor_scalar(
                out=t1s, in0=t1s, scalar1=W_DVE, scalar2=None,
                op0=mybir.AluOpType.mult,
            )
            nc.vector.scalar_tensor_tensor(
                out=mexp[:, :, :, 1], in0=astot[:, :, :, 1],
                scalar=1.0 / float(N), in1=t1s,
                op0=mybir.AluOpType.mult, op1=mybir.AluOpType.add,
            )

            scale_c = stat.tile([P, 2, 2], F32)
            shift_c = stat.tile([P, 2, 2], F32)
            tv8 = stat.tile([P, 2, 2, 16], F8)
            bv8 = stat.tile([P, 2, 2, 16], F8)
            bqe = stat.tile([P, 2, 2], F32)
            bke = stat.tile([P, 2, 2], F32)
            bve = stat.tile([P, 2, 2], F32)
            bpe = stat.tile([P, 2, 2], F32)
            neg2 = stat.tile([P, 1], F32)
            nc.vector.memset(neg2, EBIAS)

            with tc.tile_pool(name="psum_p1", bufs=1, space="PSUM") as p1:
                gs_ps = p1.tile([NGROUPS, 2], F32, tag="gs")
                kk = 0
                for g in range(2):
                    for t2 in range(2):
                        nc.tensor.matmul(
                            gs_ps, gmat[:, g, t2, :], mexp[:, g, t2, :],
                            start=(kk == 0), stop=(kk == 3),
                        )
                        kk += 1
                gsb = stat.tile([NGROUPS, 2], F32)
                nc.vector.tensor_copy(out=gsb, in_=gs_ps)
                gmr = stat.tile([NGROUPS, 2], F32)
                gtmp = stat.tile([NGROUPS, 2], F32)
                nc.vector.tensor_scalar(
                    out=gmr[:, 0:1], in0=gsb[:, 0:1], scalar1=1.0 / GSIZE,
                    scalar2=None, op0=mybir.AluOpType.mult,
                )
                nc.vector.tensor_scalar(
                    out=gtmp[:, 0:1], in0=gsb[:, 1:2], scalar1=1.0 / GSIZE,
                    scalar2=None, op0=mybir.AluOpType.mult,
                )
                nc.vector.tensor_tensor(
                    out=gtmp[:, 1:2], in0=gmr[:, 0:1], in1=gmr[:, 0:1],
                    op=mybir.AluOpType.mult,
                )
                nc.vector.tensor_sub(
                    out=gtmp[:, 0:1], in0=gtmp[:, 0:1], in1=gtmp[:, 1:2]
                )
                eps_sb = stat.tile([NGROUPS, 1], F32)
                nc.vector.memset(eps_sb, EPS)
                nc.scalar.activation(
                    out=gtmp[:, 0:1], in_=gtmp[:, 0:1],
                    func=AF.Sqrt, bias=eps_sb,
                )
                nc.vector.reciprocal(out=gmr[:, 1:2], in_=gtmp[:, 0:1])
                mc = stat.tile([P, 2, 2, 2], F32)
                ms_list = []
                for g in range(2):
                    for t2 in range(2):
                        ms_ps = p1.tile(
                            [P, 2], F32, tag="ms", bufs=4, name=f"ms{g}{t2}"
                        )
                        nc.tensor.matmul(
                            ms_ps, emat[:, g, t2, :], gmr, start=True, stop=True
                        )
                        ms_list.append((g, t2, ms_ps))
                for g, t2, ms_ps in ms_list:
                    nc.vector.tensor_copy(out=mc[:, g, t2, :], in_=ms_ps)
                nc.vector.tensor_tensor(
                    out=scale_c, in0=mc[:, :, :, 1], in1=gam_sb,
                    op=mybir.AluOpType.mult,
                )
                nc.vector.tensor_tensor(
                    out=shift_c, in0=mc[:, :, :, 0], in1=scale_c,
                    op=mybir.AluOpType.mult,
                )
                nc.vector.tensor_sub(out=shift_c, in0=bet_sb, in1=shift_c)

                # fold norm scale into q/k/v weights (wq on DVE first so the
                # q-projection can start; wk/wv on ACT in parallel)
                nc.vector.tensor_scalar(
                    out=tv8[:, :, :, 0], in0=shift_c, scalar1=TS, scalar2=None,
                    op0=mybir.AluOpType.mult,
                )
                for g in range(2):
                    for t2 in range(2):
                        sc1 = scale_c[:, g, t2 : t2 + 1]
                        if t2 == 0:
                            nc.vector.tensor_scalar(
                                out=w3f[:, g, t2, :, :], in0=w38[:, g, t2, :, :],
                                scalar1=sc1, scalar2=None,
                                op0=mybir.AluOpType.mult,
                            )
                        else:
                            nc.scalar.activation(
                                out=w3f[:, g, t2, :, :], in_=w38[:, g, t2, :, :],
                                func=AF.Identity, scale=sc1,
                            )

                # effective biases: bX_eff = wX @ shift + bX (tiny DR matvecs)
                def matvec(wi, rhs8, bh, bdst):
                    for ct in range(4):
                        g2, tt = ct // 2, ct % 2
                        be_ps = p1.tile([P, 1], F32, tag="bias", bufs=3)
                        for g in range(2):
                            lhs = (
                                wp8[:, g, :, ct * P : (ct + 1) * P]
                                if wi == 3
                                else w38[:, g, :, wi, ct * P : (ct + 1) * P]
                            )
                            nc.tensor.matmul(
                                be_ps, lhs, rhs8[:, g, :, 0:1],
                                start=(g == 0), stop=(g == 1),
                                perf_mode=DR,
                            )
                        nc.vector.tensor_scalar(
                            out=bdst[:, g2, tt : tt + 1], in0=be_ps,
                            scalar1=1.0 / TS, scalar2=bh[:, g2, tt : tt + 1],
                            op0=mybir.AluOpType.mult, op1=mybir.AluOpType.add,
                        )

                matvec(0, tv8, bqh, bqe)
                matvec(1, tv8, bkh, bke)


            # ---------------- Phase 2: q/k/v projections --------------------
            ev = {"n": 0}

            def evac(dst, src_ps, bias_ap=None):
                use_act = ev["n"] % 2 == 0
                ev["n"] += 1
                if use_act:
                    nc.scalar.activation(
                        out=dst, in_=src_ps, func=AF.Identity,
                        bias=bias_ap if bias_ap is not None else 0.0,
                    )
                elif bias_ap is None:
                    nc.vector.tensor_copy(out=dst, in_=src_ps)
                else:
                    nc.vector.tensor_scalar(
                        out=dst, in0=src_ps, scalar1=bias_ap, scalar2=None,
                        op0=mybir.AluOpType.add,
                    )

            with tc.tile_pool(name="psum2", bufs=1, space="PSUM") as p2:

                def late_biases():
                    matvec2(2, tv8, bvh, bve)
                    nc.vector.tensor_scalar(
                        out=bv8[:, :, :, 0], in0=bve, scalar1=TS, scalar2=None,
                        op0=mybir.AluOpType.mult,
                    )
                    matvec2(3, bv8, bph, bpe)

                def matvec2(wi, rhs8, bh, bdst):
                    for ct in range(4):
                        g2, tt = ct // 2, ct % 2
                        be_ps = p2.tile([P, 1], F32, tag="bias", bufs=2)
                        for g in range(2):
                            lhs = (
                                wp8[:, g, :, ct * P : (ct + 1) * P]
                                if wi == 3
                                else w38[:, g, :, wi, ct * P : (ct + 1) * P]
                            )
                            nc.tensor.matmul(
                                be_ps, lhs, rhs8[:, g, :, 0:1],
                                start=(g == 0), stop=(g == 1),
                                perf_mode=DR,
                            )
                        nc.vector.tensor_scalar(
                            out=bdst[:, g2, tt : tt + 1], in0=be_ps,
                            scalar1=1.0 / TS, scalar2=bh[:, g2, tt : tt + 1],
                            op0=mybir.AluOpType.mult, op1=mybir.AluOpType.add,
                        )

                for s in range(NS):
                    sl = slice(s * 512, (s + 1) * 512)
                    if s < 2:
                        for ct in range(4):
                            g2, tt = ct // 2, ct % 2
                            qp = p2.tile([P, 512], F32, tag="acc", bufs=3)
                            for g in range(2):
                                nc.tensor.matmul(
                                    qp,
                                    w3f[:, g, :, 0, ct * P : (ct + 1) * P],
                                    X8[:, g, :, sl],
                                    start=(g == 0), stop=(g == 1),
                                    perf_mode=DR,
                                )
                            evac(QO8[:, g2, tt, sl], qp, bqe[:, g2, tt : tt + 1])
                    for jt4 in range(4):
                        jt = s * 4 + jt4
                        jb = slice(jt * P, (jt + 1) * P)
                        vp = p2.tile([P, 512], F32, tag="acc", bufs=3)
                        for g in range(2):
                            nc.tensor.matmul(
                                vp, X8[:, g, :, jb], w3f[:, g, :, 2, :],
                                start=(g == 0), stop=(g == 1),
                                perf_mode=DR,
                            )
                        evac(VT8[:, jt // 2, jt % 2, :], vp)
                    for ct in range(4):
                        g2, tt = ct // 2, ct % 2
                        kp = p2.tile([P, 512], F32, tag="acc", bufs=3)
                        for g in range(2):
                            nc.tensor.matmul(
                                kp,
                                w3f[:, g, :, 1, ct * P : (ct + 1) * P],
                                X8[:, g, :, sl],
                                start=(g == 0), stop=(g == 1),
                                perf_mode=DR,
                            )
                        evac(K8[:, g2, tt, sl], kp, bke[:, g2, tt : tt + 1])
                    if s == 0:
                        late_biases()
                        nc.scalar.activation(out=dscr, in_=dummy, func=AF.Exp)

            # ---------------- Phase 3: attention -----------------------------
            # isl 0's output projection + epilogue are interleaved into
            # isl 1's jp loop (PSUM tag "zb" hosts zbc then the pr tiles).
            deferred = []

            def pop_deferred():
                if deferred:
                    deferred.pop(0)()

            with tc.tile_pool(name="psum3", bufs=1, space="PSUM") as p3:

                def proj_epilogue(isl, ct, O8, zbcS, p3=p3):
                    g2, tt = ct // 2, ct % 2
                    isl_sl = slice(isl * 512, (isl + 1) * 512)
                    tag = "zb" if isl == 0 else f"o{ct}"
                    pr = p3.tile([P, 512], F32, tag=tag, bufs=1, name=f"pr{isl}{ct}")
                    for g in range(2):
                        nc.tensor.matmul(
                            pr,
                            wp8[:, g, :, ct * P : (ct + 1) * P],
                            O8[:, g, :, :],
                            start=(g == 0), stop=(g == 1),
                            perf_mode=DR,
                        )
                    tno = work.tile([P, 512], F32, tag="tno", bufs=3)
                    nc.vector.tensor_tensor(
                        out=tno, in0=pr, in1=zbcS, op=mybir.AluOpType.mult,
                    )
                    nc.vector.scalar_tensor_tensor(
                        out=ost_sb[:, g2, tt, isl_sl], in0=tno,
                        scalar=bpe[:, g2, tt : tt + 1],
                        in1=xq_sb[:, g2, tt, isl_sl],
                        op0=mybir.AluOpType.add, op1=mybir.AluOpType.add,
                    )
                    nc.sync.dma_start(
                        out=OUT_d[:, g2, tt, isl_sl],
                        in_=ost_sb[:, g2, tt, isl_sl],
                    )

                for isl in range(2):
                    isl_sl = slice(isl * 512, (isl + 1) * 512)
                    o_ps = [
                        p3.tile([P, 512], F32, tag=f"o{ct}", bufs=1,
                                name=f"o{ct}_{isl}")
                        for ct in range(4)
                    ]
                    z_ps = p3.tile([P, 512], F32, tag="z", bufs=1)
                    for jp in range(NJP):
                        ptp = work.tile([P, 2, 512], F8, tag="pt", bufs=3)
                        for t2 in range(2):
                            jt = 2 * jp + t2
                            sp = p3.tile([P, 512], F32, tag="s", bufs=2)
                            for g in range(2):
                                nc.tensor.matmul(
                                    sp,
                                    K8[:, g, :, jt * P : (jt + 1) * P],
                                    QO8[:, g, :, isl_sl],
                                    start=(g == 0), stop=(g == 1),
                                    perf_mode=DR,
                                )
                            nc.scalar.activation(
                                out=ptp[:, t2, :], in_=sp,
                                func=AF.Exp, scale=SCL, bias=neg2,
                            )
                        nc.tensor.matmul(
                            z_ps, on8, ptp,
                            start=(jp == 0), stop=(jp == NJP - 1),
                            perf_mode=DR,
                        )
                        for ct in range(4):
                            nc.tensor.matmul(
                                o_ps[ct],
                                VT8[:, jp, :, ct * P : (ct + 1) * P],
                                ptp,
                                start=(jp == 0), stop=(jp == NJP - 1),
                                perf_mode=DR,
                            )
                        if jp >= 3:
                            pop_deferred()
                    # O evac on ACT (x0.25 range guard); 4/Z folded into zbcS
                    O8 = work.tile([P, 2, 2, 512], F8, tag="o8", bufs=2)
                    for ct in range(4):
                        nc.scalar.activation(
                            out=O8[:, ct // 2, ct % 2, :], in_=o_ps[ct],
                            func=AF.Identity, scale=0.25,
                        )
                    zrow = work.tile([1, 512], F32R, tag="zrow", bufs=2)
                    nc.vector.tensor_scalar(
                        out=zrow, in0=z_ps[0:1, :], scalar1=0.25, scalar2=None,
                        op0=mybir.AluOpType.mult,
                    )
                    zbc_ps = p3.tile([P, 512], F32, tag="zb", bufs=1,
                                     name=f"zbc{isl}")
                    nc.tensor.matmul(zbc_ps, onq, zrow, start=True, stop=True)
                    zbcS = work.tile([P, 512], F32, tag="zbs", bufs=2)
                    nc.vector.reciprocal(out=zbcS, in_=zbc_ps)
                    if isl == 0:
                        for ct in range(4):
                            deferred.append(
                                lambda ct=ct, O8=O8, zbcS=zbcS:
                                proj_epilogue(0, ct, O8, zbcS)
                            )
                    else:
                        for ct in range(4):
                            proj_epilogue(1, ct, O8, zbcS)
                while deferred:
                    pop_deferred()


    nc.compile()
    return nc


def _get_nc():
    if "nc" not in _cached:
        _cached["nc"] = _build_program()
    return _cached["nc"]


def _f8(a):
    return np.clip(np.ascontiguousarray(a, dtype=np.float32), -240, 240).astype(E4)


def _gt(v):
    """[C] -> [P, 2, 2] with channel c = p + 128*t + 256*g at [p, g, t]."""
    return np.ascontiguousarray(
        np.asarray(v, np.float32).reshape(2, 2, P).transpose(2, 0, 1)
    )


def _xprep(a2d, ncols):
    """[C, ncols] -> [P, 2, 2, ncols]."""
    return np.ascontiguousarray(
        a2d.reshape(2, 2, P, ncols).transpose(2, 0, 1, 3)
    )


def _wprep(w):
    """[Cout, Cin] -> lhsT layout [P, 2, 2, Cout] fp8 (ci = p+128t+256g)."""
    return _f8(np.asarray(w, np.float32).T.reshape(2, 2, P, C).transpose(2, 0, 1, 3))


def _make_in_maps(x, norm_gamma, norm_beta, wq, bq, wk, bk, wv, bv, wp, bp):
    gm = np.zeros((P, 2, 2, NGROUPS), np.float32)
    em = np.zeros((NGROUPS, 2, 2, P), np.float32)
    for g in range(2):
        for t2 in range(2):
            for p in range(P):
                grp = p // GSIZE + 8 * t2 + 16 * g
                gm[p, g, t2, grp] = 1.0
                em[grp, g, t2, p] = 1.0

    cpack = np.stack(
        [_gt(bq), _gt(bk), _gt(bv), _gt(bp), _gt(norm_gamma), _gt(norm_beta)],
        axis=-1,
    )

    common = {
        "w38": np.ascontiguousarray(
            np.stack([_wprep(wq), _wprep(wk), _wprep(wv)], axis=3)
        ),
        "wp8": _wprep(wp),
        "cpack": np.ascontiguousarray(cpack),
        "gmat": gm,
        "emat": em,
        "ones8": np.ones((P, 2, P), np.float32).astype(E4),
        "onesq": np.ones((1, P), np.float32),
    }

    in_maps = []
    for c in range(NCORES):
        b, qi = c // 4, c % 4
        xb = np.ascontiguousarray(np.asarray(x[b], dtype=np.float32).reshape(C, N))
        xp = np.concatenate([xb[:, qi * NQ :], xb[:, : qi * NQ]], axis=1)
        m = dict(common)
        m["xin8"] = _f8(_xprep(xp, N))
        m["xq"] = _xprep(xb[:, qi * NQ : (qi + 1) * NQ], NQ)
        in_maps.append(m)
    return in_maps


def _assemble(results):
    out = np.empty((B, C, N), np.float32)
    for c in range(NCORES):
        b, qi = c // 4, c % 4
        r = results[c]["out"]  # [P, 2, 2, NQ]
        out[b, :, qi * NQ : (qi + 1) * NQ] = (
            r.transpose(1, 2, 0, 3).reshape(C, NQ)
        )
    return out.reshape(B, C, HW, HW)


def _run(inputs, trace=False, trace_kwargs=None):
    nc = _get_nc()
    in_maps = _make_in_maps(**inputs)
    res = run_bass_kernel_spmd(
        nc, in_maps, list(range(NCORES)), trace=trace,
        **(trace_kwargs or {}),
    )
    return res


def kernel(**inputs):
    res = _run(inputs)
    return _assemble(res.results)


# revision 3
# speedup vs baseline: 1.1136x; 1.0772x over previous
"""AttnBlock (B=2, C=512, H=W=64) on 8 TRN2 NeuronCores — fp8 DoubleRow.

Sharding: core c handles batch b=c//4 and query-quarter q=c%4 (1024 of 4096
query positions). Keys/values are computed redundantly per core from the
full batch image; the key axis is host-permuted per core so the core's
query quarter occupies columns 0:1024 (softmax is permutation-invariant
over keys).

All matmuls run in fp8e4 (TRN FP8_EXP4, max 240) with DoubleRow perf mode:
each instruction contracts K=256 (two 128-row subtiles interleaved in the
free dim) in the same 512 cycles a bf16/f32r K=128 matmul takes — 2x.
Channel dim C=512 is stored pair-split as [128 p, 2 g, 2 t] with
c = p + 128*t + 256*g; a DR matmul contracts (p, t) for fixed g, and the
two g-groups accumulate in PSUM.

GroupNorm: bn_stats on the fp8 x (24 slices on DVE, 8 slices via ACT
Identity/Square accum_out), group reduce via one-hot matmuls; scale/shift
fold into the q/k/v weights on device (wq' = wq*diag(s), bias' = wq@t + b
via tiny DR matvecs), so no separate normalize pass exists. Softmax runs
unshifted with exp(s*C^-.5 - 2) to keep fp8 in range; Z comes from a
ones-stationary DR matmul accumulated alongside PV; O is normalized by
1/Z (broadcast via a K=1 outer-product matmul + full-width reciprocal)
during PSUM evacuation. The residual path stays exact fp32.
"""

import numpy as np
import ml_dtypes

import concourse.bass as bass
import concourse.tile as tile
from concourse import bacc, mybir
from concourse.bass_utils import run_bass_kernel_spmd

F32 = mybir.dt.float32
F32R = mybir.dt.float32r
F8 = mybir.dt.float8e4
E4 = ml_dtypes.float8_e4m3
DR = mybir.MatmulPerfMode.DoubleRow
AF = mybir.ActivationFunctionType

P = 128
C = 512
N = 4096          # H*W keys
NQ = 1024         # query columns per core
NS = 8            # 512-wide column slices of N
SPL = 6           # slices per (g,t) whose stats run on DVE (rest on ACT)
NJP = 16          # 256-wide key pair-tiles
B = 2
HW = 64
NGROUPS = 32
GSIZE = C // NGROUPS
EPS = 1e-5
SCL = float(C) ** -0.5
EBIAS = -2.0      # exp(s*SCL - 2): max logit ~5.5 -> exp(3.5)=33 << 240
TS = 64.0         # shift vector pre-scale for fp8 matvec
NCORES = 8

_cached = {}


def _build_program():
    nc = bacc.Bacc("TRN2", target_bir_lowering=False, debug=False)

    X8_d = nc.declare_dram_parameter("xin8", [P, 2, 2, N], F8, isOutput=False)
    W3_d = nc.declare_dram_parameter("w38", [P, 2, 2, 3, C], F8, isOutput=False)
    WP_d = nc.declare_dram_parameter("wp8", [P, 2, 2, C], F8, isOutput=False)
    # packed per-channel f32 consts: bq, bk, bv, bp, gamma, beta
    CP_d = nc.declare_dram_parameter("cpack", [P, 2, 2, 6], F32, isOutput=False)
    G_d = nc.declare_dram_parameter("gmat", [P, 2, 2, NGROUPS], F32, isOutput=False)
    E_d = nc.declare_dram_parameter("emat", [NGROUPS, 2, 2, P], F32, isOutput=False)
    ON8_d = nc.declare_dram_parameter("ones8", [P, 2, P], F8, isOutput=False)
    ONQ_d = nc.declare_dram_parameter("onesq", [1, P], F32R, isOutput=False)
    XQ_d = nc.declare_dram_parameter("xq", [P, 2, 2, NQ], F32, isOutput=False)
    OUT_d = nc.declare_dram_parameter("out", [P, 2, 2, NQ], F32, isOutput=True)

    with tile.TileContext(nc) as tc:
        with (
            tc.tile_pool(name="big", bufs=1) as big,
            tc.tile_pool(name="consts", bufs=1) as consts,
            tc.tile_pool(name="stat", bufs=1) as stat,
            tc.tile_pool(name="work", bufs=1) as work,
        ):
            X8 = big.tile([P, 2, 2, N], F8)
            K8 = big.tile([P, 2, 2, N], F8)
            VT8 = big.tile([P, NJP, 2, C], F8)
            QO8 = big.tile([P, 2, 2, NQ], F8)
            xq_sb = big.tile([P, 2, 2, NQ], F32)
            ost_sb = big.tile([P, 2, 2, NQ], F32)

            w38 = consts.tile([P, 2, 2, 3, C], F8)
            w3f = consts.tile([P, 2, 2, 3, C], F8)
            wp8 = consts.tile([P, 2, 2, C], F8)
            cpk = consts.tile([P, 2, 2, 6], F32)
            gmat = consts.tile([P, 2, 2, NGROUPS], F32)
            emat = consts.tile([NGROUPS, 2, 2, P], F32)
            on8 = consts.tile([P, 2, P], F8)
            onq = consts.tile([1, P], F32R)

            # preload ACT tables (Identity/Square/Sqrt/Exp) while DMA runs
            dummy = stat.tile([1, 2], F32)
            nc.vector.memset(dummy, 1.0)
            dscr = stat.tile([1, 2], F32)
            for fn in (AF.Identity, AF.Square, AF.Sqrt, AF.Exp):
                nc.scalar.activation(out=dscr, in_=dummy, func=fn)

            # x first: groupnorm stats are the serial head of the kernel
            for g in range(2):
                for t2 in range(2):
                    nc.sync.dma_start(out=X8[:, g, t2, :], in_=X8_d[:, g, t2, :])
            for t_ in (
                (w38, W3_d), (cpk, CP_d),
                (gmat, G_d), (emat, E_d), (on8, ON8_d), (onq, ONQ_d),
                (wp8, WP_d),
            ):
                nc.gpsimd.dma_start(out=t_[0], in_=t_[1][:])
            nc.gpsimd.dma_start(out=xq_sb, in_=XQ_d[:])

            bqh = cpk[:, :, :, 0]
            bkh = cpk[:, :, :, 1]
            bvh = cpk[:, :, :, 2]
            bph = cpk[:, :, :, 3]
            gam_sb = cpk[:, :, :, 4]
            bet_sb = cpk[:, :, :, 5]

            # ---------------- Phase 1: group-norm statistics ----------------
            # 24 slices via DVE bn_stats, 8 slices via ACT accum (sum, sumsq)
            bnst = stat.tile([P, 2, 2, SPL, 6], F32)
            asum = stat.tile([P, 2, 2, 2, 2], F32)
            ascr = stat.tile([P, 2, 512], F8)
            mex = stat.tile([P, 2, 2, 2], F32)
            for g in range(2):
                for t2 in range(2):
                    for s in range(SPL):
                        nc.vector.bn_stats(
                            out=bnst[:, g, t2, s, :],
                            in_=X8[:, g, t2, s * 512 : (s + 1) * 512],
                        )
                    nc.vector.bn_aggr(
                        out=mex[:, g, t2, :], in_=bnst[:, g, t2, :, :]
                    )
                    for si in range(2):
                        sl = slice((SPL + si) * 512, (SPL + si + 1) * 512)
                        nc.scalar.activation(
                            out=ascr[:, 0, :], in_=X8[:, g, t2, sl],
                            func=AF.Identity,
                            accum_out=asum[:, g, t2, si, 0:1],
                        )
                        nc.scalar.activation(
                            out=ascr[:, 1, :], in_=X8[:, g, t2, sl],
                            func=AF.Square,
                            accum_out=asum[:, g, t2, si, 1:2],
                        )
            # mexp[...,0] = mean over 4096, mexp[...,1] = E[x^2] over 4096
            W_DVE = SPL / float(NS)
            astot = stat.tile([P, 2, 2, 2], F32)
            nc.vector.tensor_add(
                out=astot, in0=asum[:, :, :, 0, :], in1=asum[:, :, :, 1, :]
            )
            mexp = stat.tile([P, 2, 2, 2], F32)
            t1s = stat.tile([P, 2, 2], F32)
            nc.vector.tensor_scalar(
                out=t1s, in0=mex[:, :, :, 0], scalar1=W_DVE, scalar2=None,
                op0=mybir.AluOpType.mult,
            )
            nc.vector.scalar_tensor_tensor(
                out=mexp[:, :, :, 0], in0=astot[:, :, :, 0],
                scalar=1.0 / float(N), in1=t1s,
                op0=mybir.AluOpType.mult, op1=mybir.AluOpType.add,
            )
            nc.vector.tensor_tensor(
                out=t1s, in0=mex[:, :, :, 0], in1=mex[:, :, :, 0],
                op=mybir.AluOpType.mult,
            )
            nc.vector.tensor_add(out=t1s, in0=t1s, in1=mex[:, :, :, 1])
            nc.vector.tensor_scalar(
                out=t1s, in0=t1s, scalar1=W_DVE, scalar2=None,
                op0=mybir.AluOpType.mult,
            )
            nc.vector.scalar_tensor_tensor(
                out=mexp[:, :, :, 1], in0=astot[:, :, :, 1],
                scalar=1.0 / float(N), in1=t1s,
                op0=mybir.AluOpType.mult, op1=mybir.AluOpType.add,
            )

            scale_c = stat.tile([P, 2, 2], F32)
            shift_c = stat.tile([P, 2, 2], F32)
            tv8 = stat.tile([P, 2, 2, 16], F8)
            bv8 = stat.tile([P, 2, 2, 16], F8)
            bqe = stat.tile([P, 2, 2], F32)
            bke = stat.tile([P, 2, 2], F32)
            bve = stat.tile([P, 2, 2], F32)
            bpe = stat.tile([P, 2, 2], F32)
            neg2 = stat.tile([P, 1], F32)
            nc.vector.memset(neg2, EBIAS)

            with tc.tile_pool(name="psum_p1", bufs=1, space="PSUM") as p1:
                gs_ps = p1.tile([NGROUPS, 2], F32, tag="gs")
                kk = 0
                for g in range(2):
                    for t2 in range(2):
                        nc.tensor.matmul(
                            gs_ps, gmat[:, g, t2, :], mexp[:, g, t2, :],
                            start=(kk == 0), stop=(kk == 3),
                        )
                        kk += 1
                gsb = stat.tile([NGROUPS, 2], F32)
                nc.vector.tensor_copy(out=gsb, in_=gs_ps)
                gmr = stat.tile([NGROUPS, 2], F32)
                gtmp = stat.tile([NGROUPS, 2], F32)
                nc.vector.tensor_scalar(
                    out=gmr[:, 0:1], in0=gsb[:, 0:1], scalar1=1.0 / GSIZE,
                    scalar2=None, op0=mybir.AluOpType.mult,
                )
                nc.vector.tensor_scalar(
                    out=gtmp[:, 0:1], in0=gsb[:, 1:2], scalar1=1.0 / GSIZE,
                    scalar2=None, op0=mybir.AluOpType.mult,
                )
                nc.vector.tensor_tensor(
                    out=gtmp[:, 1:2], in0=gmr[:, 0:1], in1=gmr[:, 0:1],
                    op=mybir.AluOpType.mult,
                )
                nc.vector.tensor_sub(
                    out=gtmp[:, 0:1], in0=gtmp[:, 0:1], in1=gtmp[:, 1:2]
                )
                eps_sb = stat.tile([NGROUPS, 1], F32)
                nc.vector.memset(eps_sb, EPS)
                nc.scalar.activation(
                    out=gtmp[:, 0:1], in_=gtmp[:, 0:1],
                    func=AF.Sqrt, bias=eps_sb,
                )
                nc.vector.reciprocal(out=gmr[:, 1:2], in_=gtmp[:, 0:1])
                mc = stat.tile([P, 2, 2, 2], F32)
                ms_list = []
                for g in range(2):
                    for t2 in range(2):
                        ms_ps = p1.tile(
                            [P, 2], F32, tag="ms", bufs=4, name=f"ms{g}{t2}"
                        )
                        nc.tensor.matmul(
                            ms_ps, emat[:, g, t2, :], gmr, start=True, stop=True
                        )
                        ms_list.append((g, t2, ms_ps))
                for g, t2, ms_ps in ms_list:
                    nc.vector.tensor_copy(out=mc[:, g, t2, :], in_=ms_ps)
                nc.vector.tensor_tensor(
                    out=scale_c, in0=mc[:, :, :, 1], in1=gam_sb,
                    op=mybir.AluOpType.mult,
                )
                nc.vector.tensor_tensor(
                    out=shift_c, in0=mc[:, :, :, 0], in1=scale_c,
                    op=mybir.AluOpType.mult,
                )
                nc.vector.tensor_sub(out=shift_c, in0=bet_sb, in1=shift_c)

                # fold norm scale into q/k/v weights (wq on DVE first so the
                # q-projection can start; wk/wv on ACT in parallel)
                nc.vector.tensor_scalar(
                    out=tv8[:, :, :, 0], in0=shift_c, scalar1=TS, scalar2=None,
                    op0=mybir.AluOpType.mult,
                )
                for g in range(2):
                    for t2 in range(2):
                        sc1 = scale_c[:, g, t2 : t2 + 1]
                        if t2 == 0:
                            nc.vector.tensor_scalar(
                                out=w3f[:, g, t2, :, :], in0=w38[:, g, t2, :, :],
                                scalar1=sc1, scalar2=None,
                                op0=mybir.AluOpType.mult,
                            )
                        else:
                            nc.scalar.activation(
                                out=w3f[:, g, t2, :, :], in_=w38[:, g, t2, :, :],
                                func=AF.Identity, scale=sc1,
                            )

                # effective biases: bX_eff = wX @ shift + bX (tiny DR matvecs)
                def matvec(wi, rhs8, bh, bdst):
                    for ct in range(4):
                        g2, tt = ct // 2, ct % 2
                        be_ps = p1.tile([P, 1], F32, tag="bias", bufs=3)
                        for g in range(2):
                            lhs = (
                                wp8[:, g, :, ct * P : (ct + 1) * P]
                                if wi == 3
                                else w38[:, g, :, wi, ct * P : (ct + 1) * P]
                            )
                            nc.tensor.matmul(
                                be_ps, lhs, rhs8[:, g, :, 0:1],
                                start=(g == 0), stop=(g == 1),
                                perf_mode=DR,
                            )
                        nc.vector.tensor_scalar(
                            out=bdst[:, g2, tt : tt + 1], in0=be_ps,
                            scalar1=1.0 / TS, scalar2=bh[:, g2, tt : tt + 1],
                            op0=mybir.AluOpType.mult, op1=mybir.AluOpType.add,
                        )

                matvec(0, tv8, bqh, bqe)
                matvec(1, tv8, bkh, bke)


            # ---------------- Phase 2: q/k/v projections --------------------
            ev = {"n": 0}

            def evac(dst, src_ps, bias_ap=None):
                use_act = ev["n"] % 2 == 0
                ev["n"] += 1
                if use_act:
                    nc.scalar.activation(
                        out=dst, in_=src_ps, func=AF.Identity,
                        bias=bias_ap if bias_ap is not None else 0.0,
                    )
                elif bias_ap is None:
                    nc.vector.tensor_copy(out=dst, in_=src_ps)
                else:
                    nc.vector.tensor_scalar(
                        out=dst, in0=src_ps, scalar1=bias_ap, scalar2=None,
                        op0=mybir.AluOpType.add,
                    )

            with tc.tile_pool(name="psum2", bufs=1, space="PSUM") as p2:

                def late_biases():
                    matvec2(2, tv8, bvh, bve)
                    nc.vector.tensor_scalar(
                        out=bv8[:, :, :, 0], in0=bve, scalar1=TS, scalar2=None,
                        op0=mybir.AluOpType.mult,
                    )
                    matvec2(3, bv8, bph, bpe)

                def matvec2(wi, rhs8, bh, bdst):
                    for ct in range(4):
                        g2, tt = ct // 2, ct % 2
                        be_ps = p2.tile([P, 1], F32, tag="bias", bufs=2)
                        for g in range(2):
                            lhs = (
                                wp8[:, g, :, ct * P : (ct + 1) * P]
                                if wi == 3
                                else w38[:, g, :, wi, ct * P : (ct + 1) * P]
                            )
                            nc.tensor.matmul(
                                be_ps, lhs, rhs8[:, g, :, 0:1],
                                start=(g == 0), stop=(g == 1),
                                perf_mode=DR,
                            )
                        nc.vector.tensor_scalar(
                            out=bdst[:, g2, tt : tt + 1], in0=be_ps,
                            scalar1=1.0 / TS, scalar2=bh[:, g2, tt : tt + 1],
                            op0=mybir.AluOpType.mult, op1=mybir.AluOpType.add,
                        )

                for s in range(NS):
                    sl = slice(s * 512, (s + 1) * 512)
                    if s < 2:
                        for ct in range(4):
                            g2, tt = ct // 2, ct % 2
                            qp = p2.tile([P, 512], F32, tag="acc", bufs=3)
                            for g in range(2):
                                nc.tensor.matmul(
                                    qp,
                                    w3f[:, g, :, 0, ct * P : (ct + 1) * P],
                                    X8[:, g, :, sl],
                                    start=(g == 0), stop=(g == 1),
                                    perf_mode=DR,
                                )
                            evac(QO8[:, g2, tt, sl], qp, bqe[:, g2, tt : tt + 1])
                    for jt4 in range(4):
                        jt = s * 4 + jt4
                        jb = slice(jt * P, (jt + 1) * P)
                        vp = p2.tile([P, 512], F32, tag="acc", bufs=3)
                        for g in range(2):
                            nc.tensor.matmul(
                                vp, X8[:, g, :, jb], w3f[:, g, :, 2, :],
                                start=(g == 0), stop=(g == 1),
                                perf_mode=DR,
                            )
                        evac(VT8[:, jt // 2, jt % 2, :], vp)
                    for ct in range(4):
                        g2, tt = ct // 2, ct % 2
                        kp = p2.tile([P, 512], F32, tag="acc", bufs=3)
                        for g in range(2):
                            nc.tensor.matmul(
                                kp,
                                w3f[:, g, :, 1, ct * P : (ct + 1) * P],
                                X8[:, g, :, sl],
                                start=(g == 0), stop=(g == 1),
                                perf_mode=DR,
                            )
                        evac(K8[:, g2, tt, sl], kp, bke[:, g2, tt : tt + 1])
                    if s == 0:
                        late_biases()
                        nc.scalar.activation(out=dscr, in_=dummy, func=AF.Exp)

            # ---------------- Phase 3: attention -----------------------------
            # isl 0's output projection + epilogue are interleaved into
            # isl 1's jp loop (PSUM tag "zb" hosts zbc then the pr tiles).
            deferred = []

            def pop_deferred():
                if deferred:
                    deferred.pop(0)()

            with tc.tile_pool(name="psum3", bufs=1, space="PSUM") as p3:

                def proj_epilogue(isl, ct, O8, zbcS, p3=p3):
                    g2, tt = ct // 2, ct % 2
                    isl_sl = slice(isl * 512, (isl + 1) * 512)
                    tag = "zb" if isl == 0 else f"o{ct}"
                    pr = p3.tile([P, 512], F32, tag=tag, bufs=1, name=f"pr{isl}{ct}")
                    for g in range(2):
                        nc.tensor.matmul(
                            pr,
                            wp8[:, g, :, ct * P : (ct + 1) * P],
                            O8[:, g, :, :],
                            start=(g == 0), stop=(g == 1),
                            perf_mode=DR,
                        )
                    tno = work.tile([P, 512], F32, tag="tno", bufs=3)
                    nc.vector.tensor_tensor(
                        out=tno, in0=pr, in1=zbcS, op=mybir.AluOpType.mult,
                    )
                    nc.vector.scalar_tensor_tensor(
                        out=ost_sb[:, g2, tt, isl_sl], in0=tno,
                        scalar=bpe[:, g2, tt : tt + 1],
                        in1=xq_sb[:, g2, tt, isl_sl],
                        op0=mybir.AluOpType.add, op1=mybir.AluOpType.add,
                    )
                    nc.gpsimd.dma_start(
                        out=OUT_d[:, g2, tt, isl_sl],
                        in_=ost_sb[:, g2, tt, isl_sl],
                    )

                for isl in range(2):
                    isl_sl = slice(isl * 512, (isl + 1) * 512)
                    o_ps = [
                        p3.tile([P, 512], F32, tag=f"o{ct}", bufs=1,
                                name=f"o{ct}_{isl}")
                        for ct in range(4)
                    ]
                    z_ps = p3.tile([P, 512], F32, tag="z", bufs=1)
                    for jp in range(NJP):
                        ptp = work.tile([P, 2, 512], F8, tag="pt", bufs=3)
                        for t2 in range(2):
                            jt = 2 * jp + t2
                            sp = p3.tile([P, 512], F32, tag="s", bufs=2)
                            for g in range(2):
                                nc.tensor.matmul(
                                    sp,
                                    K8[:, g, :, jt * P : (jt + 1) * P],
                                    QO8[:, g, :, isl_sl],
                                    start=(g == 0), stop=(g == 1),
                                    perf_mode=DR,
                                )
                            nc.scalar.activation(
                                out=ptp[:, t2, :], in_=sp,
                                func=AF.Exp, scale=SCL, bias=neg2,
                            )
                        nc.tensor.matmul(
                            z_ps, on8, ptp,
                            start=(jp == 0), stop=(jp == NJP - 1),
                            perf_mode=DR,
                        )
                        for ct in range(4):
                            nc.tensor.matmul(
                                o_ps[ct],
                                VT8[:, jp, :, ct * P : (ct + 1) * P],
                                ptp,
                                start=(jp == 0), stop=(jp == NJP - 1),
                                perf_mode=DR,
                            )
                        if jp >= 1:
                            pop_deferred()
                    # x0.25 range guard on O/Z; 4/Z folded into zbcS.
                    # isl0: evac on DVE, zbc/recip/prs deferred into isl1's
                    # jp loop so the in-order PE stream never stalls on them.
                    zrow = work.tile([1, 512], F32R, tag="zrow", bufs=2)
                    nc.vector.tensor_scalar(
                        out=zrow, in0=z_ps[0:1, :], scalar1=0.25, scalar2=None,
                        op0=mybir.AluOpType.mult,
                    )
                    O8 = work.tile([P, 2, 2, 512], F8, tag="o8", bufs=2)
                    zbcS = work.tile([P, 512], F32, tag="zbs", bufs=2,
                                     name=f"zbcS{isl}")

                    def emit_zbc(isl=isl, zrow=zrow, zbcS=zbcS):
                        zbc_ps = p3.tile([P, 512], F32, tag="zb", bufs=1,
                                         name=f"zbc{isl}")
                        nc.tensor.matmul(zbc_ps, onq, zrow, start=True, stop=True)
                        nc.vector.reciprocal(out=zbcS, in_=zbc_ps)

                    if isl == 0:
                        for ct in range(4):
                            nc.vector.tensor_scalar(
                                out=O8[:, ct // 2, ct % 2, :], in0=o_ps[ct],
                                scalar1=0.25, scalar2=None,
                                op0=mybir.AluOpType.mult,
                            )
                        deferred.append(emit_zbc)
                        for ct in range(4):
                            deferred.append(
                                lambda ct=ct, O8=O8, zbcS=zbcS:
                                proj_epilogue(0, ct, O8, zbcS)
                            )
                    else:
                        for ct in range(4):
                            nc.scalar.activation(
                                out=O8[:, ct // 2, ct % 2, :], in_=o_ps[ct],
                                func=AF.Identity, scale=0.25,
                            )
                        emit_zbc()
                        for ct in range(4):
                            proj_epilogue(1, ct, O8, zbcS)
                while deferred:
                    pop_deferred()


    nc.compile()
    return nc


def _get_nc():
    if "nc" not in _cached:
        _cached["nc"] = _build_program()
    return _cached["nc"]


def _f8(a):
    return np.clip(np.ascontiguousarray(a, dtype=np.float32), -240, 240).astype(E4)


def _gt(v):
    """[C] -> [P, 2, 2] with channel c = p + 128*t + 256*g at [p, g, t]."""
    return np.ascontiguousarray(
        np.asarray(v, np.float32).reshape(2, 2, P).transpose(2, 0, 1)
    )


def _xprep(a2d, ncols):
    """[C, ncols] -> [P, 2, 2, ncols]."""
    return np.ascontiguousarray(
        a2d.reshape(2, 2, P, ncols).transpose(2, 0, 1, 3)
    )


def _wprep(w):
    """[Cout, Cin] -> lhsT layout [P, 2, 2, Cout] fp8 (ci = p+128t+256g)."""
    return _f8(np.asarray(w, np.float32).T.reshape(2, 2, P, C).transpose(2, 0, 1, 3))


def _make_in_maps(x, norm_gamma, norm_beta, wq, bq, wk, bk, wv, bv, wp, bp):
    gm = np.zeros((P, 2, 2, NGROUPS), np.float32)
    em = np.zeros((NGROUPS, 2, 2, P), np.float32)
    for g in range(2):
        for t2 in range(2):
            for p in range(P):
                grp = p // GSIZE + 8 * t2 + 16 * g
                gm[p, g, t2, grp] = 1.0
                em[grp, g, t2, p] = 1.0

    cpack = np.stack(
        [_gt(bq), _gt(bk), _gt(bv), _gt(bp), _gt(norm_gamma), _gt(norm_beta)],
        axis=-1,
    )

    common = {
        "w38": np.ascontiguousarray(
            np.stack([_wprep(wq), _wprep(wk), _wprep(wv)], axis=3)
        ),
        "wp8": _wprep(wp),
        "cpack": np.ascontiguousarray(cpack),
        "gmat": gm,
        "emat": em,
        "ones8": np.ones((P, 2, P), np.float32).astype(E4),
        "onesq": np.ones((1, P), np.float32),
    }

    in_maps = []
    for c in range(NCORES):
        b, qi = c // 4, c % 4
        xb = np.ascontiguousarray(np.asarray(x[b], dtype=np.float32).reshape(C, N))
        xp = np.concatenate([xb[:, qi * NQ :], xb[:, : qi * NQ]], axis=1)
        m = dict(common)
        m["xin8"] = _f8(_xprep(xp, N))
        m["xq"] = _xprep(xb[:, qi * NQ : (qi + 1) * NQ], NQ)
        in_maps.append(m)
    return in_maps


def _assemble(results):
    out = np.empty((B, C, N), np.float32)
    for c in range(NCORES):
        b, qi = c // 4, c % 4
        r = results[c]["out"]  # [P, 2, 2, NQ]
        out[b, :, qi * NQ : (qi + 1) * NQ] = (
            r.transpose(1, 2, 0, 3).reshape(C, NQ)
        )
    return out.reshape(B, C, HW, HW)


def _run(inputs, trace=False, trace_kwargs=None):
    nc = _get_nc()
    in_maps = _make_in_maps(**inputs)
    res = run_bass_kernel_spmd(
        nc, in_maps, list(range(NCORES)), trace=trace,
        **(trace_kwargs or {}),
    )
    return res


def kernel(**inputs):
    res = _run(inputs)
    return _assemble(res.results)
